# revision 1
# baseline (speedup 1.0000x reference)
"""Trainium2 Bass kernel for the quirky-reshape 16-head attention layer.

Shapes (hardcoded): x [2, 2048, 1024], Wq/Wk/Wv/Wo [1024, 1024], n_head=16.

Sharding: core c in [0,8) handles batch b=c//4 and head group g=c%4 (heads
4g..4g+3). The reference's quirky `qkv.reshape(b, s, d)` merge makes output
rows [h*128, (h+1)*128) depend only on head h, so each core produces the
disjoint output row block [g*512, (g+1)*512) of its batch — no collectives.

Precision: q/k path (projections + scores) in fp16, exp / AV / O-projection
in bf16 (fp32 range needed: exp values reach ~1e30), all matmul accumulation
in fp32 PSUM. End-to-end scale-relative absmax error ~3.8e-3.

Per-core dataflow (transposed-scores streaming attention):
  qT = Wq^T x^T [128, 2048] fp16 pair tiles (2 heads x 64 rows); kT is
  stored as per-head [128, 2048] tiles with the other head's rows zeroed so
  every score matmul is full-array K=128 (K=64 matmuls never un-throttle the
  PE HAM clock gate and run at 1.2 GHz instead of 2.4).
  V[kb] [128, 4, 128] bf16: per-head blocks [ones(64) | v(64)].
  per (head, 1024-wide q chunk), streaming over 16 key blocks kb:
    S^T[kb] = kTz_h[:,kb]^T qT     -> PSUM [128, 1024] fp32
    E[kb]   = exp(S^T[kb])         -> ScalarE, SBUF bf16   (ACT-bound phase)
    AV[half] += [1|v]^T E[kb]      -> PSUM [128, 512] x2, one kb behind the
                                      exp pipeline so the PE never stalls
  AV rows 0:64 = softmax denominator broadcast, rows 64:128 = qkv unnorm.
  rcp = reciprocal_approx_fast(denom); DMA rcp to partitions 64:128;
  Q2[64:128, t, h, u] = qkv * rcp (bf16, t-major layout so O-projection
  reads are contiguous); Q2[0:64, odd t] = DMA shift of even t.
  out_h [128, 1024] = sum_kt Q2[:, 2kt+1, h, :]^T Wo[kt]  (natural
  orientation; Wo is the moving operand, so each Q2 stationary serves 2
  matmuls).
  Projection and O-projection matmul groups are interleaved into the
  attention loops as PSUM-slot-sized fillers to overlap phases.
"""

import numpy as np

B, S, D, H = 2, 2048, 1024, 16
DH = 64
NCORES = 8

_CACHE = {}


def _build_program():
    from concourse import bacc, tile, mybir

    F32 = mybir.dt.float32
    F16 = mybir.dt.float16
    BF16 = mybir.dt.bfloat16
    EXP = mybir.ActivationFunctionType.Exp

    nc = bacc.Bacc(None, target_bir_lowering=False, debug=False)

    xt_d = nc.dram_tensor("xt", [128, 8, 2048], F16, kind="ExternalInput").ap()
    wq_d = nc.dram_tensor("wq", [128, 8, 256], F16, kind="ExternalInput").ap()
    wk_d = nc.dram_tensor("wk", [128, 8, 256], F16, kind="ExternalInput").ap()
    wv_d = nc.dram_tensor("wv", [128, 8, 256], F16, kind="ExternalInput").ap()
    wo_d = nc.dram_tensor("wo", [128, 8, 1024], BF16, kind="ExternalInput").ap()
    out_d = nc.dram_tensor("out", [4, 128, 1024], F32, kind="ExternalOutput").ap()

    with tile.TileContext(nc) as tc:
        with (
            tc.tile_pool(name="keep", bufs=1) as keep,
            tc.tile_pool(name="exp", bufs=8) as expp,
            tc.tile_pool(name="rcp", bufs=3) as rcpp,
            tc.tile_pool(name="osb", bufs=2) as osbp,
            tc.tile_pool(name="ps", bufs=1, space="PSUM") as psp,
        ):
            # ------- input DMAs (xt split across 3 queues) -------
            xt = []
            wq = []
            wk = []
            wv = []
            for kt in range(8):
                t = keep.tile([128, 2048], F16, tag=f"xt{kt}", name=f"xt{kt}")
                eng = (nc.sync, nc.scalar, nc.gpsimd)[kt % 3]
                eng.dma_start(out=t[:], in_=xt_d[:, kt, :])
                xt.append(t)
            for nm, lst, dram in (("wq", wq, wq_d), ("wk", wk, wk_d), ("wv", wv, wv_d)):
                for kt in range(8):
                    t = keep.tile([128, 256], F16, tag=f"{nm}{kt}", name=f"{nm}{kt}")
                    eng = nc.sync if kt % 2 == 0 else nc.scalar
                    eng.dma_start(out=t[:], in_=dram[:, kt, :])
                    lst.append(t)
            wo = []
            for kt in range(8):
                t = keep.tile([128, 1024], BF16, tag=f"wo{kt}", name=f"wo{kt}")
                nc.gpsimd.dma_start(out=t[:], in_=wo_d[:, kt, :])
                wo.append(t)
            q2 = keep.tile([128, 16, 4, 128], BF16, tag="q2")

            # ------- persistent result tiles + emit helpers -------
            v_sb = {}
            for kb in range(16):
                vt = keep.tile([128, 4, 128], BF16, tag=f"v{kb}", name=f"v{kb}")
                v_sb[kb] = vt
                nc.vector.memset(vt[:], 1.0)
            qk_sb = {}
            for pair in range(2):
                t = keep.tile([128, 2048], F16, tag=f"qT{pair}", name=f"qT{pair}")
                qk_sb[("q", pair)] = t
                tA = keep.tile([128, 2048], F16, tag=f"kTz{pair}0", name=f"kTz{pair}0")
                tB = keep.tile([128, 2048], F16, tag=f"kTz{pair}1", name=f"kTz{pair}1")
                nc.vector.memset(tA[64:128, :], 0.0)
                nc.vector.memset(tB[0:64, :], 0.0)
                qk_sb[("k", pair, 0)] = tA
                qk_sb[("k", pair, 1)] = tB

            def v_group(kb):
                vt = v_sb[kb]
                ps = psp.tile([128, 256], F32, tag="sc", bufs=2, name="vps")
                for kt in range(8):
                    nc.tensor.matmul(
                        ps[:],
                        xt[kt][:, kb * 128:(kb + 1) * 128],
                        wv[kt][:],
                        start=(kt == 0),
                        stop=(kt == 7),
                    )
                nc.vector.tensor_copy(vt[:, :, 64:128],
                                      ps[:].rearrange("p (a b) -> p a b", a=4))

            def qk_group(nm, pair, ch):
                wt = wq if nm == "q" else wk
                ps = psp.tile([128, 512], F32, tag="sc", bufs=2, name="qkps")
                for kt in range(8):
                    nc.tensor.matmul(
                        ps[:],
                        wt[kt][:, pair * 128:(pair + 1) * 128],
                        xt[kt][:, ch * 512:(ch + 1) * 512],
                        start=(kt == 0),
                        stop=(kt == 7),
                    )
                cs = slice(ch * 512, (ch + 1) * 512)
                if nm == "q":
                    nc.vector.tensor_copy(qk_sb[("q", pair)][:, cs], ps[:])
                else:
                    nc.vector.tensor_copy(qk_sb[("k", pair, 0)][0:64, cs], ps[0:64, :])
                    nc.vector.tensor_copy(qk_sb[("k", pair, 1)][64:128, cs], ps[64:128, :])

            def oproj_group(hg, nh=None):
                # natural-orientation O-projection; each q2-slice stationary
                # serves both 512-wide halves (2 matmuls per weight load)
                ops = {h: psp.tile([128, 512], F32, tag="sc", bufs=2, name="ops")
                       for h in range(2)}
                for kt in range(8):
                    for h in range(2):
                        nc.tensor.matmul(
                            ops[h][:],
                            q2[:, 2 * kt + 1, hg, :],
                            wo[kt][:, h * 512:(h + 1) * 512],
                            start=(kt == 0),
                            stop=(kt == 7),
                        )
                for h in range(2):
                    ot = osbp.tile([128, 512], F32, tag="ot", name="ot")
                    nc.vector.tensor_copy(ot[:], ops[h][:])
                    nc.sync.dma_start(out=out_d[hg, :, h * 512:(h + 1) * 512], in_=ot[:])

            def attn(pair, fillers):
                qT = qk_sb[("q", pair)]
                kTz = {0: qk_sb[("k", pair, 0)], 1: qk_sb[("k", pair, 1)]}
                fi = 0
                it = 0
                for qc in range(2):
                    av = {}
                    for hl in range(2):
                        for half in range(2):
                            av[(hl, half)] = psp.tile([128, 512], F32, tag="av", bufs=4, name="av")

                    def av_mms(kbp, et_prev):
                        for hl in range(2):
                            hg = 2 * pair + hl
                            lhsT = v_sb[kbp][:, hg, :]
                            for half in range(2):
                                nc.tensor.matmul(
                                    av[(hl, half)][:],
                                    lhsT,
                                    et_prev[hl][:, half * 512:(half + 1) * 512],
                                    start=(kbp == 0),
                                    stop=(kbp == 15),
                                )

                    prev_et = None
                    for kb in range(16):
                        while fi < len(fillers) and fillers[fi][0] <= it:
                            fillers[fi][1]()
                            fi += 1
                        it += 1
                        sc = {}
                        for hl in range(2):
                            sc[hl] = psp.tile([128, 1024], F32, tag="sc", bufs=2, name="sc")
                        for hl in range(2):
                            for sub in range(2):
                                q0 = qc * 1024 + sub * 512
                                nc.tensor.matmul(
                                    sc[hl][:, sub * 512:(sub + 1) * 512],
                                    kTz[hl][:, kb * 128:(kb + 1) * 128],
                                    qT[:, q0:q0 + 512],
                                    start=True,
                                    stop=True,
                                )
                        # software-pipelined AV: consume exp of kb-1 so the
                        # PE never waits on ScalarE inside an iteration
                        if prev_et is not None:
                            av_mms(kb - 1, prev_et)
                        et = {}
                        for hl in range(2):
                            et[hl] = expp.tile([128, 1024], BF16, tag="exp", name="et")
                            nc.scalar.activation(et[hl][:], sc[hl][:], EXP)
                        prev_et = et
                    av_mms(15, prev_et)

                    for hl in range(2):
                        hg = 2 * pair + hl
                        for half in range(2):
                            ap = av[(hl, half)]
                            rt = rcpp.tile([128, 512], F32, tag="rcp", name="rt")
                            nc.vector.reciprocal_approx_fast(rt[0:64, :], ap[0:64, :])
                            nc.sync.dma_start(out=rt[64:128, :], in_=rt[0:64, :])
                            q0 = qc * 1024 + half * 512
                            u0 = q0 // 16
                            dst = q2[64:128, :, hg, u0:u0 + 32].transpose([0, 2, 1])
                            nc.vector.tensor_mul(
                                dst,
                                ap[64:128, :].rearrange("p (u t) -> p u t", t=16),
                                rt[64:128, :].rearrange("p (u t) -> p u t", t=16),
                            )
                while fi < len(fillers):
                    fillers[fi][1]()
                    fi += 1
                # shift-copy the lower 64 partitions for this pair's heads
                for hl in range(2):
                    hg = 2 * pair + hl
                    nc.sync.dma_start(
                        out=q2[0:64, 1::2, hg, :], in_=q2[64:128, 0::2, hg, :]
                    )

            # ------- pre-attention minimum: qT0 chunks 0/1, kTz0 chunk 0, V 0/1
            qk_group("q", 0, 0)
            qk_group("q", 0, 1)
            qk_group("k", 0, 0)
            v_group(0)
            v_group(1)

            # fillers for attn0: remaining V tiles + kTz0/qT0 chunks
            f0 = []
            f0.append((0, lambda: v_group(2)))
            f0.append((0, lambda: v_group(3)))
            f0.append((1, lambda: v_group(4)))
            f0.append((1, lambda: qk_group("k", 0, 1)))
            f0.append((2, lambda: v_group(5)))
            f0.append((3, lambda: v_group(6)))
            f0.append((4, lambda: v_group(7)))
            f0.append((5, lambda: qk_group("k", 0, 2)))
            f0.append((6, lambda: v_group(8)))
            f0.append((7, lambda: v_group(9)))
            f0.append((8, lambda: v_group(10)))
            f0.append((9, lambda: qk_group("k", 0, 3)))
            f0.append((10, lambda: v_group(11)))
            f0.append((11, lambda: v_group(12)))
            f0.append((12, lambda: v_group(13)))
            f0.append((13, lambda: v_group(14)))
            f0.append((14, lambda: v_group(15)))
            f0.append((15, lambda: qk_group("q", 0, 2)))
            f0.append((16, lambda: qk_group("q", 0, 3)))
            attn(0, f0)

            # pair boundary: minimum to start attn1 (kTz1 ch0 + qT1 qc0)
            qk_group("k", 1, 0)
            qk_group("q", 1, 0)
            qk_group("q", 1, 1)

            # fillers for attn1: remaining qk1 chunks only
            f1 = []
            f1.append((1, lambda: qk_group("k", 1, 1)))
            f1.append((5, lambda: qk_group("k", 1, 2)))
            f1.append((9, lambda: qk_group("k", 1, 3)))
            f1.append((12, lambda: qk_group("q", 1, 2)))
            f1.append((13, lambda: qk_group("q", 1, 3)))
            attn(1, f1)

            # tail: output projections
            for hg in range(4):
                oproj_group(hg)

    nc.compile()
    return nc


def _get_program():
    if "nc" not in _CACHE:
        _CACHE["nc"] = _build_program()
    return _CACHE["nc"]


def _make_in_maps(x, Wq, Wk, Wv, Wo):
    import ml_dtypes

    bf16 = ml_dtypes.bfloat16
    wo8 = np.ascontiguousarray(
        Wo.astype(bf16).reshape(8, 128, 1024).transpose(1, 0, 2))
    xts = [
        np.ascontiguousarray(
            x[b].T.astype(np.float16).reshape(8, 128, 2048).transpose(1, 0, 2))
        for b in range(B)
    ]
    wq16 = Wq.astype(np.float16)
    wk16 = Wk.astype(np.float16)
    wv16 = Wv.astype(np.float16)
    def pack(w, cols):
        return np.ascontiguousarray(
            w[:, cols].reshape(8, 128, 256).transpose(1, 0, 2))
    in_maps = []
    for c in range(NCORES):
        b, g = c // 4, c % 4
        cols = slice(4 * g * DH, 4 * (g + 1) * DH)
        in_maps.append(
            {
                "xt": xts[b],
                "wq": pack(wq16, cols),
                "wk": pack(wk16, cols),
                "wv": pack(wv16, cols),
                "wo": wo8,
            }
        )
    return in_maps


def kernel(x, Wq, Wk, Wv, Wo, n_head):
    from concourse.bass_utils import run_bass_kernel_spmd

    assert int(n_head) == H
    x = np.asarray(x, np.float32)
    Wq = np.asarray(Wq, np.float32)
    Wk = np.asarray(Wk, np.float32)
    Wv = np.asarray(Wv, np.float32)
    Wo = np.asarray(Wo, np.float32)

    nc = _get_program()
    in_maps = _make_in_maps(x, Wq, Wk, Wv, Wo)
    res = run_bass_kernel_spmd(nc, in_maps, list(range(NCORES)))

    out = np.empty((B, S, D), np.float32)
    for c in range(NCORES):
        b, g = c // 4, c % 4
        out[b, g * 512:(g + 1) * 512, :] = res.results[c]["out"].reshape(512, 1024)
    return out

